# revision 30
# baseline (speedup 1.0000x reference)
"""Causal self-attention (B=16, T=1024, C=768, H=12) on 8 NeuronCores.

Strategy: data-parallel over batch (2 batches per core, no collectives).

v3 redesign (from v2 trace analysis: PE busy 327us vs 172us flop-ideal,
95us of matmul at HAM half-clock, attention MMs at half array width):
  - x is transposed on the HOST and shipped as x^T tiles: kills the 96
    PE transposes (24.6us) and their 96 DVE PSUM->SBUF copies.
  - Attention S^T computed pair-concurrent: head A (dims 0-63) and head
    B (dims 64-127) issue back-to-back K=64 matmuls whose auto-derived
    tile_position puts them in disjoint PE row groups -> they execute
    concurrently (one N-pass for two heads).
  - Queries processed in 512-chunks; each (pair, chunk, key-tile) owns a
    [128, 1024] PSUM slot (head A cols 0-511, head B 512-1023) so ONE
    exp ACT with a [128, 2, w] strided AP serves both heads (halves the
    352-cycle-per-ACT ScalarE overhead) and PSUM stays in 8 banks:
    2x st double-buffer (4) + py (2) + pco (2).
  - Softmax denominators ride along as the 65th va column (ones); the l
    rows are pulled off PSUM by ScalarE copy, DMA-gathered to l_all, and
    inverted with ONE reciprocal_approx_fast per batch (v2 spent 26.5us
    in DVE RECIPROCAL).
  - Normalization pair-packed: one K=12 selector matmul broadcasts both
    heads' 1/l rows into [128, 512], one in-place DVE mul normalizes the
    stacked yraw pair tile.
  - B (next pair) / C / E / norm groups are queued as PE fillers inside
    the exp-bound attention windows so the PE never idles long enough
    for the HAM clock gate to re-throttle.
"""

import os
import numpy as np
from collections import deque
from contextlib import ExitStack

import concourse.bass as bass
import concourse.mybir as mybir
import concourse.tile as tile
from concourse.bass import ds, ts
from concourse.bass_utils import run_bass_kernel_spmd

F32 = mybir.dt.float32
F32R = mybir.dt.float32r
BF = mybir.dt.bfloat16

B, T, C, H = 16, 1024, 768, 12
D = C // H           # 64
NCORES = 8
B_LOC = B // NCORES  # 2
KT = C // 128        # 6 contraction tiles
TT = T // 128        # 8 token tiles
NPAIR = H // 2       # 6 head pairs
QW = 512             # query chunk width
EXP = mybir.ActivationFunctionType.Exp
CPY = mybir.ActivationFunctionType.Copy
LN = mybir.ActivationFunctionType.Ln


def split_multi_waits(nc):
    """Hoist surplus sync waits onto standalone EventSemaphore instructions.

    The walrus build in this environment rejects any instruction carrying
    more than one sync wait ("Too many sync wait commands"). Engine queues
    execute in order, so waiting on each semaphore in a preceding
    EventSemaphore instruction is equivalent to waiting on all of them at
    the original instruction.
    """
    n_split = 0
    for f in nc.m.functions:
        for blk in f.blocks:
            out = []
            for inst in blk.instructions:
                si = inst.sync_info
                if si is not None and si.on_wait and len(si.on_wait) > 1:
                    waits = list(si.on_wait)
                    for w in waits[:-1]:
                        n_split += 1
                        ev = mybir.InstEventSemaphore(
                            name=f"I-waitsplit-{n_split}",
                            ins=[],
                            outs=[],
                            engine=inst.engine,
                            sync_info=mybir.SyncInfo(on_wait=[w], on_update=[]),
                        )
                        out.append(ev)
                    si.on_wait = waits[-1:]
                out.append(inst)
            blk.instructions[:] = out
    return n_split


def build_program(split_waits=True, level=None):
    """split_waits: apply the multi-wait splitting (required for neuronx-cc
    codegen, but the CoreSim race detector rejects the synthetic
    EventSemaphore instructions — pass False when simulating)."""
    if level is None:
        level = int(os.environ.get("BUILD_LEVEL", "5"))
    nc = bass.Bass()
    xt = nc.declare_dram_parameter("xt", [B_LOC, 128, KT, T], BF, isOutput=False)
    wqkv = nc.declare_dram_parameter("wqkv", [C, 3 * C], BF, isOutput=False)
    wproj = nc.declare_dram_parameter("wproj", [C, C], BF, isOutput=False)
    bqkt = nc.declare_dram_parameter("bqkt", [128, 2 * NPAIR], F32, isOutput=False)
    bvbc = nc.declare_dram_parameter("bvbc", [128, C], F32, isOutput=False)
    bobc = nc.declare_dram_parameter("bobc", [128, C], F32, isOutput=False)
    maskb = nc.declare_dram_parameter("maskb", [128, 256], BF, isOutput=False)
    sel12 = nc.declare_dram_parameter("sel12", [128, C], BF, isOutput=False)
    out = nc.declare_dram_parameter("out", [B_LOC, T, C], F32, isOutput=True)

    with tile.TileContext(nc) as tc, ExitStack() as ctx, \
            nc.allow_low_precision(reason="bf16 matmul pipeline"):
        consts = ctx.enter_context(tc.tile_pool(name="consts", bufs=1))
        wq_pool = ctx.enter_context(tc.tile_pool(name="wq", bufs=1))
        wp_pool = ctx.enter_context(tc.tile_pool(name="wp", bufs=1))
        xt_pool = ctx.enter_context(tc.tile_pool(name="xt", bufs=2))
        qk_pool = ctx.enter_context(tc.tile_pool(name="qk", bufs=4))
        va_pool = ctx.enter_context(tc.tile_pool(name="va", bufs=2))
        pexp = ctx.enter_context(tc.tile_pool(name="pexp", bufs=4))
        lpool = ctx.enter_context(tc.tile_pool(name="lpool", bufs=2))
        yraw_pool = ctx.enter_context(tc.tile_pool(name="yraw", bufs=2))
        stg_pool = ctx.enter_context(tc.tile_pool(name="stg", bufs=3))
        ostage = ctx.enter_context(tc.tile_pool(name="ostage", bufs=3))
        # PSUM: st 2x2 + py 1 + lp 1 + pco 2x1 = 8 banks
        st_pool = ctx.enter_context(tc.tile_pool(name="st", bufs=2, space="PSUM"))
        py_pool = ctx.enter_context(tc.tile_pool(name="py", bufs=1, space="PSUM"))
        lp_pool = ctx.enter_context(tc.tile_pool(name="lp", bufs=1, space="PSUM"))
        pco = ctx.enter_context(tc.tile_pool(name="pco", bufs=2, space="PSUM"))

        # ---- constants -------------------------------------------------
        mask_sb = consts.tile([128, 256], BF)
        nc.gpsimd.dma_start(mask_sb[:], maskb[:])
        bqk_sb = consts.tile([128, 2 * NPAIR], F32)
        bvbc_sb = consts.tile([128, C], F32)
        bobc_sb = consts.tile([128, C], F32)
        sel_sb = consts.tile([128, C], BF)
        ones_sb = consts.tile([128, 1], BF)
        nc.vector.memset(ones_sb[:], 1.0)

        # ---- weights: bf16 from host; one strided DMA per tensor -------
        wqall = wq_pool.tile([128, KT, 3 * C], BF, name="wqall")
        wpall = wp_pool.tile([128, KT, C], BF, name="wpall")
        wq = [wqall[:, k, :] for k in range(KT)]
        wp = [wpall[:, k, :] for k in range(KT)]

        def emit_weight_dmas(stage):
            """stage 0: what C0/B0(0) need soon; stage 1: the rest.
            The sync DMA queue is in-order, so order by first use."""
            wqkv3b = wqkv.rearrange("(k p) c -> p k c", p=128)
            if stage == 0:
                nc.sync.dma_start(wqall[:, :, 0:384],
                                  wqkv3[:, :, 2 * C : 2 * C + 384])
                nc.sync.dma_start(wqall[:, :, 384:],
                                  wqkv3[:, :, 2 * C + 384 :])
                nc.sync.dma_start(bvbc_sb[:], bvbc[:])
                # pair-0 q/k weight columns for the interleaved B0(0)
                nc.sync.dma_start(wqall[:, :, 0:128], wqkv3[:, :, 0:128])
                nc.sync.dma_start(wqall[:, :, C : C + 128], wqkv3[:, :, C : C + 128])
                nc.sync.dma_start(bqk_sb[:], bqkt[:])
            else:
                nc.sync.dma_start(wqall[:, :, 128:C], wqkv3[:, :, 128:C])
                nc.sync.dma_start(wqall[:, :, C + 128 : 2 * C],
                                  wqkv3[:, :, C + 128 : 2 * C])
                nc.sync.dma_start(wpall[:], wproj.rearrange("(k p) c -> p k c", p=128))
                nc.sync.dma_start(bobc_sb[:], bobc[:])
                nc.sync.dma_start(sel_sb[:], sel12[:])

        xts = [None, None]       # per-batch x^T tiles [128, KT, T]
        l5s = [None, None]       # pair-5 l rows [2, T]
        va_tiles = {}            # (b, tt) -> va tile
        cur_qk = {}              # (batch, pair) -> (qT, kT)
        yraws = {}               # (b, p) -> [128, T] bf16
        l_alls = [None, None]
        recs = [None, None]
        rec5s = [None, None]

        def emit_xt_dma(b, split_first=False):
            xs = xt_pool.tile([128, KT, T], BF, tag="xts", name="xts")
            src = xt[b]
            if split_first:
                # land the first two token tiles early so C can start
                nc.sync.dma_start(xs[:, :, 0:256], src[:, :, 0:256])
                nc.sync.dma_start(xs[:, :, 256:T], src[:, :, 256:T])
            else:
                nc.sync.dma_start(xs[:], src[:])
            xts[b] = xs

        def emit_va_setup(b):
            for tt in range(TT):
                va = va_pool.tile(
                    [128, H * (D + 1)], BF, tag=f"va{tt}", name=f"va{tt}"
                )
                va3 = va.rearrange("p (h e) -> p h e", e=D + 1)
                nc.vector.memset(va3[:, :, D : D + 1], 1.0)
                va_tiles[(b, tt)] = va

        def emit_Cgroup(b, tt, half):
            xs = xts[b]
            pv = pco.tile([128, 384], F32, tag="mm", name="pv")
            for k in range(KT):
                nc.tensor.matmul(
                    pv[:],
                    lhsT=xs[:, k, ts(tt, 128)],
                    rhs=wq[k][:, ds(384 * half, 384)],
                    start=(k == 0),
                    stop=(k == KT - 1),
                )
            va3 = va_tiles[(b, tt)].rearrange("p (h e) -> p h e", e=D + 1)
            nc.vector.tensor_add(
                va3[:, ds(6 * half, 6), 0:D],
                pv[:].rearrange("p (h e) -> p h e", e=D),
                bvbc_sb[:, ds(384 * half, 384)].rearrange("p (h e) -> p h e", e=D),
            )

        def emit_Bgroup(b, p, which, half):
            xs = xts[b]
            if (b, p) not in cur_qk:
                qT = qk_pool.tile([128, T], BF, tag="qT", name="qT")
                kTt = qk_pool.tile([128, T], BF, tag="kT", name="kTt")
                cur_qk[(b, p)] = (qT, kTt)
            dst = cur_qk[(b, p)][which]
            pq = pco.tile([128, 512], F32, tag="mm", name="pq")
            colbase = 128 * p + which * C
            for k in range(KT):
                nc.tensor.matmul(
                    pq[:],
                    lhsT=wq[k][:, ds(colbase, 128)],
                    rhs=xs[:, k, ds(512 * half, 512)],
                    start=(k == 0),
                    stop=(k == KT - 1),
                )
            j = p + which * NPAIR
            nc.vector.tensor_scalar_add(
                dst[:, ds(512 * half, 512)], pq[:], bqk_sb[:, ds(j, 1)]
            )

        pending = deque()  # items: (cost_ns, closure)

        def pop_filler(budget=1):
            # pop ~budget ns of filler work (always at least nothing/one item)
            while pending and budget > 0:
                cost, fn = pending.popleft()
                fn()
                budget -= cost

        def drain_fillers():
            while pending:
                pending.popleft()[1]()

        def emit_D_pair(b, p, qc, pop_budget=500):
            """Attention for head pair p, query columns [512qc, 512qc+512).

            Per key-tile i: two row-group-concurrent K=64 matmuls produce
            S^T for heads A/B into one [128, 1024] PSUM slot (A cols
            0-511, B cols 512-1023), one strided exp ACT serves both, the
            causal diag blocks are masked on GpSimd, then two M=65
            matmuls accumulate V^T-weighted sums + the l row into py.
            """
            qT, kTt = cur_qk[(b, p)]
            n_i = 4 * (qc + 1)
            py = py_pool.tile([65, 1024], F32, tag="py", name="py")
            sts = {}

            def est(i):
                qoff = max(0, 128 * i - QW * qc)
                w = QW - qoff
                st = st_pool.tile([128, 1024], F32, tag="st", name="st")
                for h2 in range(2):
                    nc.tensor.matmul(
                        st[:, ds(QW * h2 + qoff, w)],
                        lhsT=kTt[ds(D * h2, D), ts(i, 128)],
                        rhs=qT[ds(D * h2, D), ds(QW * qc + qoff, w)],
                        start=True,
                        stop=True,
                    )
                sts[i] = (st, qoff, w)

            est(0)
            est(1)
            for i in range(n_i):
                st, qoff, w = sts.pop(i)
                pe = pexp.tile([128, 1024], BF, tag="pe", name="pe")
                st3 = st.rearrange("p (c w) -> p c w", c=2)
                pe3 = pe.rearrange("p (c w) -> p c w", c=2)
                nc.scalar.activation(
                    pe3[:, :, ds(qoff, w)], st3[:, :, ds(qoff, w)], EXP, scale=0.125
                )
                if i >= 4 * qc:  # causal diagonal tile: mask both heads
                    diag = pe3[:, :, ds(qoff, 128)]
                    nc.gpsimd.tensor_mul(
                        diag, diag, mask_sb.rearrange("p (c w) -> p c w", c=2)
                    )
                if i + 2 < n_i:
                    est(i + 2)
                pop_filler(pop_budget)
                va = va_tiles[(b, i)]
                for h2 in range(2):
                    nc.tensor.matmul(
                        py[ds(D * h2, D), ds(qoff, w)],
                        lhsT=va[:, ds((D + 1) * (2 * p + h2), D)],
                        rhs=pe[:, ds(QW * h2 + qoff, w)],
                        start=(i == 0),
                        stop=(i == n_i - 1),
                        skip_group_check=True,
                    )
                for h2 in range(2):
                    nc.tensor.matmul(
                        lp[ds(32 * h2, 1), ds(qoff, w)],
                        lhsT=ones_sb[:, 0:1],
                        rhs=pe[:, ds(QW * h2 + qoff, w)],
                        start=(i == 0),
                        stop=(i == n_i - 1),
                        skip_group_check=True,
                    )
            # extract l rows (lp partitions 0/32) and stacked y
            lstg = stg_pool.tile([33, QW], F32, tag="lstg", name="lstg")
            nc.vector.tensor_copy(lstg[ds(0, 1), :], lp[ds(0, 1), :])
            nc.vector.tensor_copy(lstg[ds(32, 1), :], lp[ds(32, 1), :])
            if p < NPAIR - 1:
                ldst = l_alls[b]
                row = 2 * p
            else:
                ldst = l5s[b]
                row = 0
            nc.sync.dma_start(
                ldst[ds(row, 1), ds(QW * qc, QW)], lstg[ds(0, 1), :]
            )
            nc.sync.dma_start(
                ldst[ds(row + 1, 1), ds(QW * qc, QW)], lstg[ds(32, 1), :]
            )
            if (b, p) not in yraws:
                yraws[(b, p)] = yraw_pool.tile(
                    [128, T], BF, tag=f"yr{p}", name=f"yr{p}"
                )
            yr = yraws[(b, p)]
            nc.vector.tensor_copy(yr[:, ds(QW * qc, QW)], py[:])

        def emit_recip_a(b):
            # 1/l = exp(-ln l) on ScalarE: ln and exp share the
            # natural_log_exp_and_others table set (no table switch), and
            # the exp pass writes bf16 directly for the selector matmul.
            # Part a: pairs 0-4 (emittable right after pair 4's l DMAs).
            lnl = lpool.tile([10, T], F32, tag="lnl", name="lnl")
            rec_bf = lpool.tile([10, T], BF, tag="rec_bf", name="rec_bf")
            nc.scalar.activation(lnl[:], l_alls[b][:], LN)
            nc.scalar.activation(rec_bf[:], lnl[:], EXP, scale=-1.0)
            recs[b] = rec_bf

        def emit_recip_b(b, half):
            # pair 5: separate [2, T] tile so the operand base partition is
            # 0; split by query half so cols 0-511 invert right after qc0
            if rec5s[b] is None:
                rec5s[b] = (
                    lpool.tile([2, T], F32, tag="ln5", name="ln5"),
                    lpool.tile([2, T], BF, tag="rec5_bf", name="rec5_bf"),
                )
            ln5, rec5_bf = rec5s[b]
            sl = ds(QW * half, QW)
            nc.scalar.activation(ln5[:, sl], l5s[b][:, sl], LN)
            nc.scalar.activation(rec5_bf[:, sl], ln5[:, sl], EXP, scale=-1.0)

        def emit_norm(b, p, half):
            """Broadcast 1/l for both heads of pair p to 128 partitions via
            a K=12 selector matmul, then normalize the stacked yraw pair
            tile in place."""
            pbt = pco.tile([128, 512], F32, tag="mm", name="pbt")
            if p < NPAIR - 1:
                # sel rows 0-9 of block p: rows 2p/2p+1 select the pair
                sel_ap = sel_sb[ds(0, 10), ds(128 * p, 128)]
                rec_ap = recs[b][ds(0, 10), ds(512 * half, 512)]
            else:
                # sel rows 0-1 over cols 0:128 are eye2 (x) ones64
                sel_ap = sel_sb[ds(0, 2), ds(0, 128)]
                rec_ap = rec5s[b][1][ds(0, 2), ds(512 * half, 512)]
            nc.tensor.matmul(
                pbt[:], lhsT=sel_ap, rhs=rec_ap, start=True, stop=True
            )
            yr = yraws[(b, p)]
            nc.vector.tensor_mul(
                yr[:, ds(512 * half, 512)], yr[:, ds(512 * half, 512)], pbt[:]
            )

        def emit_Egroup(b, tt, half):
            po = pco.tile([128, 384], F32, tag="mm", name="po")
            for k in range(KT):
                nc.tensor.matmul(
                    po[:],
                    lhsT=yraws[(b, k)][:, ts(tt, 128)],
                    rhs=wp[k][:, ds(384 * half, 384)],
                    start=(k == 0),
                    stop=(k == KT - 1),
                )
            ot = ostage.tile([128, 384], F32, tag="ot", name="ot")
            nc.vector.tensor_add(ot[:], po[:], bobc_sb[:, ds(384 * half, 384)])
            nc.sync.dma_start(out[b, ts(tt, 128), ds(384 * half, 384)], ot[:])

        # ---- program ---------------------------------------------------
        wqkv3 = wqkv.rearrange("(k p) c -> p k c", p=128)
        xs0, x80 = emit_xt_dma(0, split_first=True)
        nc.sync.dma_start(wqall[:, :, 0:384], wqkv3[:, :, 2 * C : 2 * C + 384])
        nc.sync.dma_start(x80[:], xt8[0][:])
        nc.sync.dma_start(wqk8_sb[:, :, 0:128], wqk8[:, :, 0:128])
        nc.sync.dma_start(wqk8_sb[:, :, C : C + 128], wqk8[:, :, C : C + 128])
        nc.sync.dma_start(bqk_sb[:], bqkt[:])
        nc.sync.dma_start(bvbc_sb[:], bvbc[:])
        nc.sync.dma_start(xs0[:, :, 512:T], xt[0][:, :, 512:T])
        nc.sync.dma_start(wqall[:, :, 384:], wqkv3[:, :, 2 * C + 384 :])
        emit_weight_dmas(1)
        l_alls[0] = lpool.tile([10, T], F32, tag="l_all", name="l_all")
        l5s[0] = lpool.tile([2, T], F32, tag="l5", name="l5")

        # C0 with B0(pair 0) interleaved
        emit_va_setup(0)
        g = 0
        for half in range(2):
            for tt in range(TT):
                emit_Cgroup(0, tt, half)
                if tt in (3, 7):
                    emit_Bgroup(0, 0, g // 2, g % 2)
                    g += 1

        if level <= 2:
            for tt in range(TT):
                nc.sync.dma_start(
                    out[0, ts(tt, 128), 0:390], va_tiles[(0, tt)][:, :].bitcast(F32)
                )
            qT, kTt = cur_qk[0]
            nc.sync.dma_start(out[0, 0:128, 390:646], qT[:, 0:512].bitcast(F32))
        else:
            # D0: attention batch 0; fillers = B0(p+1), then xt1/C1/B1(0)
            for p in range(NPAIR):
                if p < NPAIR - 1:
                    for g in reversed(range(4)):
                        pending.appendleft(
                            (800, lambda g=g, pn=p + 1: emit_Bgroup(0, pn, g // 2, g % 2))
                        )
                if p == 0 and level >= 5:
                    # batch-1 inputs early: C1 groups fill D0 pairs 1-4
                    # (fp8 shrank the B fillers below the exp-window need)
                    l_alls[1] = lpool.tile([10, T], F32, tag="l_all", name="l_all")
                    l5s[1] = lpool.tile([2, T], F32, tag="l5", name="l5")
                    pending.append((200, lambda: emit_xt_dma(1)))
                    pending.append((300, lambda: emit_va_setup(1)))
                if p == 2 and level >= 5:
                    for tt in range(TT):
                        for half in range(2):
                            pending.append(
                                (1100, lambda tt=tt, half=half: emit_Cgroup(1, tt, half))
                            )
                emit_D_pair(0, p, 0)
                emit_D_pair(0, p, 1)
                if p == NPAIR - 2:
                    # pairs 0-4 l rows have landed; reciprocal them now so
                    # norm0(p<5) can fill pair 5's exp window, and B1(0,1)
                    # give pair 5 matmul filler
                    emit_recip_a(0)
                    for pn in range(NPAIR - 1):
                        pending.append((350, lambda pn=pn: emit_norm(0, pn, 0)))
                    for pn in range(NPAIR - 1):
                        pending.append((350, lambda pn=pn: emit_norm(0, pn, 1)))
                    if level >= 5:
                        for g in range(4):
                            pending.append(
                                (800, lambda g=g: emit_Bgroup(1, 0, g // 2, g % 2))
                            )
                        for g in range(4):
                            pending.append(
                                (800, lambda g=g: emit_Bgroup(1, 1, g // 2, g % 2))
                            )
            drain_fillers()
            emit_recip_b(0, 0)
            emit_recip_b(0, 1)

            if level <= 4:
                emit_norm(0, NPAIR - 1, 0)
                emit_norm(0, NPAIR - 1, 1)
                for p in range(NPAIR):
                    nc.sync.dma_start(
                        out[0, ts(p, 128), 0:512], yraws[(0, p)][:, :].bitcast(F32)
                    )
                nc.sync.dma_start(
                    out[0, ds(6 * 128, 12), 0:768], l_alls[0][:, 0:768]
                )
            else:
                # D1 fillers: norm0(5)/E0 queued up front, B1(p+1) per pair,
                # norm1 + early E1 in the pair-5 windows
                pending.append((350, lambda: emit_norm(0, NPAIR - 1, 0)))
                pending.append((350, lambda: emit_norm(0, NPAIR - 1, 1)))
                for tt in range(TT):
                    for half in range(2):
                        pending.append(
                            (1100, lambda tt=tt, half=half: emit_Egroup(0, tt, half))
                        )
                for p in range(NPAIR):
                    if 0 < p < NPAIR - 1:
                        # front of the queue: must drain within this pair's
                        # pops so cur_qk[p+1] exists at pair p+1 emission
                        for g in reversed(range(4)):
                            pending.appendleft(
                                (800, lambda g=g, pn=p + 1: emit_Bgroup(1, pn, g // 2, g % 2))
                            )
                    emit_D_pair(1, p, 0)
                    if p == NPAIR - 1:
                        # between pair-5 halves: invert l5 cols 0-511 so
                        # norm1(5, h0) and E1(tt 0-3) can fill qc1's window
                        emit_recip_b(1, 0)
                        pending.append((350, lambda: emit_norm(1, NPAIR - 1, 0)))
                        for tt in range(4):
                            for half in range(2):
                                pending.append(
                                    (1100, lambda tt=tt, half=half: emit_Egroup(1, tt, half))
                                )
                    emit_D_pair(1, p, 1)
                    if p == NPAIR - 2:
                        emit_recip_a(1)
                        for pn in range(NPAIR - 1):
                            pending.append((350, lambda pn=pn: emit_norm(1, pn, 0)))
                        for pn in range(NPAIR - 1):
                            pending.append((350, lambda pn=pn: emit_norm(1, pn, 1)))
                drain_fillers()
                emit_recip_b(1, 1)
                emit_norm(1, NPAIR - 1, 1)
                for tt in range(4, TT):
                    for half in range(2):
                        emit_Egroup(1, tt, half)

    if split_waits:
        split_multi_waits(nc)
    return nc


def make_in_maps(x, W_qkv, b_qkv, W_proj, b_proj):
    x = np.ascontiguousarray(np.asarray(x, dtype=np.float32))
    W_qkv = np.ascontiguousarray(np.asarray(W_qkv, dtype=np.float32))
    b_qkv = np.asarray(b_qkv, dtype=np.float32)
    W_proj = np.ascontiguousarray(np.asarray(W_proj, dtype=np.float32))
    b_proj = np.asarray(b_proj, dtype=np.float32)

    import ml_dtypes

    x16 = x.astype(ml_dtypes.bfloat16)
    wqkv16 = np.ascontiguousarray(W_qkv.astype(ml_dtypes.bfloat16))
    wproj16 = np.ascontiguousarray(W_proj.astype(ml_dtypes.bfloat16))
    bqkt = np.ascontiguousarray(b_qkv[: 2 * C].reshape(2 * NPAIR, 128).T)
    bvbc = np.ascontiguousarray(np.tile(b_qkv[2 * C :].reshape(1, C), (128, 1)))
    bobc = np.ascontiguousarray(np.tile(b_proj.reshape(1, C), (128, 1)))
    m1 = np.triu(np.ones((128, 128), dtype=np.float32))
    maskb = np.ascontiguousarray(
        np.concatenate([m1, m1], axis=1).astype(ml_dtypes.bfloat16)
    )
    # selector: rows j<12, sel[j, 64h+m] = (j == h); fp32 bits used as f32r
    sel = np.zeros((128, C), dtype=np.float32)
    for h in range(H):
        sel[h, 64 * h : 64 * (h + 1)] = 1.0
    sel = sel.astype(ml_dtypes.bfloat16)

    shared = {
        "wqkv": wqkv16,
        "wproj": wproj16,
        "bqkt": bqkt,
        "bvbc": bvbc,
        "bobc": bobc,
        "maskb": maskb,
        "sel12": sel,
    }
    in_maps = []
    for c in range(NCORES):
        m = dict(shared)
        xc = x16[B_LOC * c : B_LOC * (c + 1)]  # [B_LOC, T, C]
        # xt[b, p, k, t] = x[b, t, 128k+p]
        xtc = xc.transpose(0, 2, 1).reshape(B_LOC, KT, 128, T).transpose(0, 2, 1, 3)
        m["xt"] = np.ascontiguousarray(xtc)
        in_maps.append(m)
    return in_maps


_PROGRAM = None


def kernel(x, W_qkv, b_qkv, W_proj, b_proj):
    global _PROGRAM
    if _PROGRAM is None:
        _PROGRAM = build_program()
    in_maps = make_in_maps(x, W_qkv, b_qkv, W_proj, b_proj)
    res = run_bass_kernel_spmd(_PROGRAM, in_maps, list(range(NCORES)))
    out = np.concatenate([res.results[c]["out"] for c in range(NCORES)], axis=0)
    return out.astype(np.float32)


if __name__ == "__main__":
    nc = build_program()
    print("built ok; instructions:", sum(len(bb.instructions) for f in nc.m.functions for bb in f.blocks))


# revision 31
# speedup vs baseline: 1.0804x; 1.0804x over previous
"""Causal self-attention (B=16, T=1024, C=768, H=12) on 8 NeuronCores.

Strategy: data-parallel over batch (2 batches per core, no collectives).

v3 redesign (from v2 trace analysis: PE busy 327us vs 172us flop-ideal,
95us of matmul at HAM half-clock, attention MMs at half array width):
  - x is transposed on the HOST and shipped as x^T tiles: kills the 96
    PE transposes (24.6us) and their 96 DVE PSUM->SBUF copies.
  - Attention S^T computed pair-concurrent: head A (dims 0-63) and head
    B (dims 64-127) issue back-to-back K=64 matmuls whose auto-derived
    tile_position puts them in disjoint PE row groups -> they execute
    concurrently (one N-pass for two heads).
  - Queries processed in 512-chunks; each (pair, chunk, key-tile) owns a
    [128, 1024] PSUM slot (head A cols 0-511, head B 512-1023) so ONE
    exp ACT with a [128, 2, w] strided AP serves both heads (halves the
    352-cycle-per-ACT ScalarE overhead) and PSUM stays in 8 banks:
    2x st double-buffer (4) + py (2) + pco (2).
  - Softmax denominators ride along as the 65th va column (ones); the l
    rows are pulled off PSUM by ScalarE copy, DMA-gathered to l_all, and
    inverted with ONE reciprocal_approx_fast per batch (v2 spent 26.5us
    in DVE RECIPROCAL).
  - Normalization pair-packed: one K=12 selector matmul broadcasts both
    heads' 1/l rows into [128, 512], one in-place DVE mul normalizes the
    stacked yraw pair tile.
  - B (next pair) / C / E / norm groups are queued as PE fillers inside
    the exp-bound attention windows so the PE never idles long enough
    for the HAM clock gate to re-throttle.
"""

import os
import numpy as np
from collections import deque
from contextlib import ExitStack

import concourse.bass as bass
import concourse.mybir as mybir
import concourse.tile as tile
from concourse.bass import ds, ts
from concourse.bass_utils import run_bass_kernel_spmd

F32 = mybir.dt.float32
F32R = mybir.dt.float32r
BF = mybir.dt.bfloat16

B, T, C, H = 16, 1024, 768, 12
D = C // H           # 64
NCORES = 8
B_LOC = B // NCORES  # 2
KT = C // 128        # 6 contraction tiles
TT = T // 128        # 8 token tiles
NPAIR = H // 2       # 6 head pairs
QW = 512             # query chunk width
EXP = mybir.ActivationFunctionType.Exp
CPY = mybir.ActivationFunctionType.Copy
LN = mybir.ActivationFunctionType.Ln


def split_multi_waits(nc):
    """Hoist surplus sync waits onto standalone EventSemaphore instructions.

    The walrus build in this environment rejects any instruction carrying
    more than one sync wait ("Too many sync wait commands"). Engine queues
    execute in order, so waiting on each semaphore in a preceding
    EventSemaphore instruction is equivalent to waiting on all of them at
    the original instruction.
    """
    n_split = 0
    for f in nc.m.functions:
        for blk in f.blocks:
            out = []
            for inst in blk.instructions:
                si = inst.sync_info
                if si is not None and si.on_wait and len(si.on_wait) > 1:
                    waits = list(si.on_wait)
                    for w in waits[:-1]:
                        n_split += 1
                        ev = mybir.InstEventSemaphore(
                            name=f"I-waitsplit-{n_split}",
                            ins=[],
                            outs=[],
                            engine=inst.engine,
                            sync_info=mybir.SyncInfo(on_wait=[w], on_update=[]),
                        )
                        out.append(ev)
                    si.on_wait = waits[-1:]
                out.append(inst)
            blk.instructions[:] = out
    return n_split


def build_program(split_waits=True, level=None):
    """split_waits: apply the multi-wait splitting (required for neuronx-cc
    codegen, but the CoreSim race detector rejects the synthetic
    EventSemaphore instructions — pass False when simulating)."""
    if level is None:
        level = int(os.environ.get("BUILD_LEVEL", "5"))
    nc = bass.Bass()
    xt = nc.declare_dram_parameter("xt", [B_LOC, 128, KT, T], BF, isOutput=False)
    wqkv = nc.declare_dram_parameter("wqkv", [C, 3 * C], BF, isOutput=False)
    wproj = nc.declare_dram_parameter("wproj", [C, C], BF, isOutput=False)
    bqkt = nc.declare_dram_parameter("bqkt", [128, 2 * NPAIR], F32, isOutput=False)
    bvbc = nc.declare_dram_parameter("bvbc", [128, C], F32, isOutput=False)
    bobc = nc.declare_dram_parameter("bobc", [128, C], F32, isOutput=False)
    maskb = nc.declare_dram_parameter("maskb", [128, 256], BF, isOutput=False)
    sel12 = nc.declare_dram_parameter("sel12", [128, C], BF, isOutput=False)
    out = nc.declare_dram_parameter("out", [B_LOC, T, C], F32, isOutput=True)

    with tile.TileContext(nc) as tc, ExitStack() as ctx, \
            nc.allow_low_precision(reason="bf16 matmul pipeline"):
        consts = ctx.enter_context(tc.tile_pool(name="consts", bufs=1))
        wq_pool = ctx.enter_context(tc.tile_pool(name="wq", bufs=1))
        wp_pool = ctx.enter_context(tc.tile_pool(name="wp", bufs=1))
        xt_pool = ctx.enter_context(tc.tile_pool(name="xt", bufs=2))
        qk_pool = ctx.enter_context(tc.tile_pool(name="qk", bufs=4))
        va_pool = ctx.enter_context(tc.tile_pool(name="va", bufs=2))
        pexp = ctx.enter_context(tc.tile_pool(name="pexp", bufs=4))
        lpool = ctx.enter_context(tc.tile_pool(name="lpool", bufs=2))
        yraw_pool = ctx.enter_context(tc.tile_pool(name="yraw", bufs=2))
        stg_pool = ctx.enter_context(tc.tile_pool(name="stg", bufs=3))
        ostage = ctx.enter_context(tc.tile_pool(name="ostage", bufs=3))
        # PSUM: st 2x2 banks + py 1x2 banks + pco 2x1 bank = 8 banks
        st_pool = ctx.enter_context(tc.tile_pool(name="st", bufs=2, space="PSUM"))
        py_pool = ctx.enter_context(tc.tile_pool(name="py", bufs=1, space="PSUM"))
        pco = ctx.enter_context(tc.tile_pool(name="pco", bufs=2, space="PSUM"))

        # ---- constants -------------------------------------------------
        mask_sb = consts.tile([128, 256], BF)
        nc.gpsimd.dma_start(mask_sb[:], maskb[:])
        bqk_sb = consts.tile([128, 2 * NPAIR], F32)
        bvbc_sb = consts.tile([128, C], F32)
        bobc_sb = consts.tile([128, C], F32)
        sel_sb = consts.tile([128, C], BF)

        # ---- weights: bf16 from host; one strided DMA per tensor -------
        wqall = wq_pool.tile([128, KT, 3 * C], BF, name="wqall")
        wpall = wp_pool.tile([128, KT, C], BF, name="wpall")
        wq = [wqall[:, k, :] for k in range(KT)]
        wp = [wpall[:, k, :] for k in range(KT)]

        def emit_weight_dmas(stage):
            """stage 0: what C0/B0(0) need soon; stage 1: the rest.
            The sync DMA queue is in-order, so order by first use."""
            wqkv3b = wqkv.rearrange("(k p) c -> p k c", p=128)
            if stage == 0:
                nc.sync.dma_start(wqall[:, :, 0:384],
                                  wqkv3[:, :, 2 * C : 2 * C + 384])
                nc.sync.dma_start(wqall[:, :, 384:],
                                  wqkv3[:, :, 2 * C + 384 :])
                nc.sync.dma_start(bvbc_sb[:], bvbc[:])
                # pair-0 q/k weight columns for the interleaved B0(0)
                nc.sync.dma_start(wqall[:, :, 0:128], wqkv3[:, :, 0:128])
                nc.sync.dma_start(wqall[:, :, C : C + 128], wqkv3[:, :, C : C + 128])
                nc.sync.dma_start(bqk_sb[:], bqkt[:])
            else:
                nc.sync.dma_start(wqall[:, :, 128:C], wqkv3[:, :, 128:C])
                nc.sync.dma_start(wqall[:, :, C + 128 : 2 * C],
                                  wqkv3[:, :, C + 128 : 2 * C])
                nc.sync.dma_start(wpall[:], wproj.rearrange("(k p) c -> p k c", p=128))
                nc.sync.dma_start(bobc_sb[:], bobc[:])
                nc.sync.dma_start(sel_sb[:], sel12[:])

        xts = [None, None]       # per-batch x^T tiles [128, KT, T]
        l5s = [None, None]       # pair-5 l rows [2, T]
        va_tiles = {}            # (b, tt) -> va tile
        cur_qk = {}              # (batch, pair) -> (qT, kT)
        yraws = {}               # (b, p) -> [128, T] bf16
        l_alls = [None, None]
        recs = [None, None]
        rec5s = [None, None]

        def emit_xt_dma(b, split_first=False):
            xs = xt_pool.tile([128, KT, T], BF, tag="xts", name="xts")
            src = xt[b]
            if split_first:
                # land the first two token tiles early so C can start
                nc.sync.dma_start(xs[:, :, 0:256], src[:, :, 0:256])
                nc.sync.dma_start(xs[:, :, 256:T], src[:, :, 256:T])
            else:
                nc.sync.dma_start(xs[:], src[:])
            xts[b] = xs

        def emit_va_setup(b):
            for tt in range(TT):
                va = va_pool.tile(
                    [128, H * (D + 1)], BF, tag=f"va{tt}", name=f"va{tt}"
                )
                va3 = va.rearrange("p (h e) -> p h e", e=D + 1)
                nc.vector.memset(va3[:, :, D : D + 1], 1.0)
                va_tiles[(b, tt)] = va

        def emit_Cgroup(b, tt, half):
            xs = xts[b]
            pv = pco.tile([128, 384], F32, tag="mm", name="pv")
            for k in range(KT):
                nc.tensor.matmul(
                    pv[:],
                    lhsT=xs[:, k, ts(tt, 128)],
                    rhs=wq[k][:, ds(384 * half, 384)],
                    start=(k == 0),
                    stop=(k == KT - 1),
                )
            va3 = va_tiles[(b, tt)].rearrange("p (h e) -> p h e", e=D + 1)
            nc.vector.tensor_add(
                va3[:, ds(6 * half, 6), 0:D],
                pv[:].rearrange("p (h e) -> p h e", e=D),
                bvbc_sb[:, ds(384 * half, 384)].rearrange("p (h e) -> p h e", e=D),
            )

        def emit_Bgroup(b, p, which, half):
            xs = xts[b]
            if (b, p) not in cur_qk:
                qT = qk_pool.tile([128, T], BF, tag="qT", name="qT")
                kTt = qk_pool.tile([128, T], BF, tag="kT", name="kTt")
                cur_qk[(b, p)] = (qT, kTt)
            dst = cur_qk[(b, p)][which]
            pq = pco.tile([128, 512], F32, tag="mm", name="pq")
            colbase = 128 * p + which * C
            for k in range(KT):
                nc.tensor.matmul(
                    pq[:],
                    lhsT=wq[k][:, ds(colbase, 128)],
                    rhs=xs[:, k, ds(512 * half, 512)],
                    start=(k == 0),
                    stop=(k == KT - 1),
                )
            j = p + which * NPAIR
            nc.vector.tensor_scalar_add(
                dst[:, ds(512 * half, 512)], pq[:], bqk_sb[:, ds(j, 1)]
            )

        pending = deque()  # items: (cost_ns, closure)

        def pop_filler(budget=1):
            # pop ~budget ns of filler work (always at least nothing/one item)
            while pending and budget > 0:
                cost, fn = pending.popleft()
                fn()
                budget -= cost

        def drain_fillers():
            while pending:
                pending.popleft()[1]()

        def emit_D_pair(b, p, qc, pop_budget=500):
            """Attention for head pair p, query columns [512qc, 512qc+512).

            Per key-tile i: two row-group-concurrent K=64 matmuls produce
            S^T for heads A/B into one [128, 1024] PSUM slot (A cols
            0-511, B cols 512-1023), one strided exp ACT serves both, the
            causal diag blocks are masked on GpSimd, then two M=65
            matmuls accumulate V^T-weighted sums + the l row into py.
            """
            qT, kTt = cur_qk[(b, p)]
            n_i = 4 * (qc + 1)
            py = py_pool.tile([65, 1024], F32, tag="py", name="py")
            sts = {}

            def est(i):
                qoff = max(0, 128 * i - QW * qc)
                w = QW - qoff
                st = st_pool.tile([128, 1024], F32, tag="st", name="st")
                for h2 in range(2):
                    nc.tensor.matmul(
                        st[:, ds(QW * h2 + qoff, w)],
                        lhsT=kTt[ds(D * h2, D), ts(i, 128)],
                        rhs=qT[ds(D * h2, D), ds(QW * qc + qoff, w)],
                        start=True,
                        stop=True,
                    )
                sts[i] = (st, qoff, w)

            est(0)
            est(1)
            for i in range(n_i):
                st, qoff, w = sts.pop(i)
                pe = pexp.tile([128, 1024], BF, tag="pe", name="pe")
                st3 = st.rearrange("p (c w) -> p c w", c=2)
                pe3 = pe.rearrange("p (c w) -> p c w", c=2)
                nc.scalar.activation(
                    pe3[:, :, ds(qoff, w)], st3[:, :, ds(qoff, w)], EXP, scale=0.125
                )
                if i >= 4 * qc:  # causal diagonal tile: mask both heads
                    diag = pe3[:, :, ds(qoff, 128)]
                    nc.gpsimd.tensor_mul(
                        diag, diag, mask_sb.rearrange("p (c w) -> p c w", c=2)
                    )
                if i + 2 < n_i:
                    est(i + 2)
                pop_filler(pop_budget)
                va = va_tiles[(b, i)]
                for h2 in range(2):
                    nc.tensor.matmul(
                        py[ds(0, 65), ds(QW * h2 + qoff, w)],
                        lhsT=va[:, ds((D + 1) * (2 * p + h2), D + 1)],
                        rhs=pe[:, ds(QW * h2 + qoff, w)],
                        start=(i == 0),
                        stop=(i == n_i - 1),
                        skip_group_check=True,
                    )
            # extract l rows (partition 64) and unnormalized y (0-63)
            lstg = stg_pool.tile([65, 1024], F32, tag="lstg", name="lstg")
            nc.vector.tensor_copy(lstg[ds(64, 1), :], py[ds(64, 1), :])
            if p < NPAIR - 1:
                ldst = l_alls[b]
                row = 2 * p
            else:
                ldst = l5s[b]
                row = 0
            nc.sync.dma_start(
                ldst[ds(row, 1), ds(QW * qc, QW)], lstg[ds(64, 1), 0:QW]
            )
            nc.sync.dma_start(
                ldst[ds(row + 1, 1), ds(QW * qc, QW)],
                lstg[ds(64, 1), QW : 2 * QW],
            )
            if (b, p) not in yraws:
                yraws[(b, p)] = yraw_pool.tile(
                    [128, T], BF, tag=f"yr{p}", name=f"yr{p}"
                )
            yr = yraws[(b, p)]
            nc.vector.tensor_copy(yr[ds(0, D), ds(QW * qc, QW)], py[ds(0, D), 0:QW])
            sb = stg_pool.tile([D, QW], BF, tag="sb", name="sb")
            nc.vector.tensor_copy(sb[:], py[ds(0, D), QW : 2 * QW])
            nc.sync.dma_start(yr[ds(D, D), ds(QW * qc, QW)], sb[:])

        def emit_recip_a(b):
            # 1/l = exp(-ln l) on ScalarE: ln and exp share the
            # natural_log_exp_and_others table set (no table switch), and
            # the exp pass writes bf16 directly for the selector matmul.
            # Part a: pairs 0-4 (emittable right after pair 4's l DMAs).
            lnl = lpool.tile([10, T], F32, tag="lnl", name="lnl")
            rec_bf = lpool.tile([10, T], BF, tag="rec_bf", name="rec_bf")
            nc.scalar.activation(lnl[:], l_alls[b][:], LN)
            nc.scalar.activation(rec_bf[:], lnl[:], EXP, scale=-1.0)
            recs[b] = rec_bf

        def emit_recip_b(b, half):
            # pair 5: separate [2, T] tile so the operand base partition is
            # 0; split by query half so cols 0-511 invert right after qc0
            if rec5s[b] is None:
                rec5s[b] = (
                    lpool.tile([2, T], F32, tag="ln5", name="ln5"),
                    lpool.tile([2, T], BF, tag="rec5_bf", name="rec5_bf"),
                )
            ln5, rec5_bf = rec5s[b]
            sl = ds(QW * half, QW)
            nc.scalar.activation(ln5[:, sl], l5s[b][:, sl], LN)
            nc.scalar.activation(rec5_bf[:, sl], ln5[:, sl], EXP, scale=-1.0)

        def emit_norm(b, p, half):
            """Broadcast 1/l for both heads of pair p to 128 partitions via
            a K=12 selector matmul, then normalize the stacked yraw pair
            tile in place."""
            pbt = pco.tile([128, 512], F32, tag="mm", name="pbt")
            if p < NPAIR - 1:
                # sel rows 0-9 of block p: rows 2p/2p+1 select the pair
                sel_ap = sel_sb[ds(0, 10), ds(128 * p, 128)]
                rec_ap = recs[b][ds(0, 10), ds(512 * half, 512)]
            else:
                # sel rows 0-1 over cols 0:128 are eye2 (x) ones64
                sel_ap = sel_sb[ds(0, 2), ds(0, 128)]
                rec_ap = rec5s[b][1][ds(0, 2), ds(512 * half, 512)]
            nc.tensor.matmul(
                pbt[:], lhsT=sel_ap, rhs=rec_ap, start=True, stop=True
            )
            yr = yraws[(b, p)]
            nc.vector.tensor_mul(
                yr[:, ds(512 * half, 512)], yr[:, ds(512 * half, 512)], pbt[:]
            )

        def emit_Egroup(b, tt, half):
            po = pco.tile([128, 384], F32, tag="mm", name="po")
            for k in range(KT):
                nc.tensor.matmul(
                    po[:],
                    lhsT=yraws[(b, k)][:, ts(tt, 128)],
                    rhs=wp[k][:, ds(384 * half, 384)],
                    start=(k == 0),
                    stop=(k == KT - 1),
                )
            ot = ostage.tile([128, 384], F32, tag="ot", name="ot")
            nc.vector.tensor_add(ot[:], po[:], bobc_sb[:, ds(384 * half, 384)])
            nc.sync.dma_start(out[b, ts(tt, 128), ds(384 * half, 384)], ot[:])

        # ---- program ---------------------------------------------------
        wqkv3 = wqkv.rearrange("(k p) c -> p k c", p=128)
        xs0, x80 = emit_xt_dma(0, split_first=True)
        nc.sync.dma_start(wqall[:, :, 0:384], wqkv3[:, :, 2 * C : 2 * C + 384])
        nc.sync.dma_start(x80[:], xt8[0][:])
        nc.sync.dma_start(wqk8_sb[:, :, 0:128], wqk8[:, :, 0:128])
        nc.sync.dma_start(wqk8_sb[:, :, C : C + 128], wqk8[:, :, C : C + 128])
        nc.sync.dma_start(bqk_sb[:], bqkt[:])
        nc.sync.dma_start(bvbc_sb[:], bvbc[:])
        nc.sync.dma_start(xs0[:, :, 512:T], xt[0][:, :, 512:T])
        nc.sync.dma_start(wqall[:, :, 384:], wqkv3[:, :, 2 * C + 384 :])
        emit_weight_dmas(1)
        l_alls[0] = lpool.tile([10, T], F32, tag="l_all", name="l_all")
        l5s[0] = lpool.tile([2, T], F32, tag="l5", name="l5")

        # C0 with B0(pair 0) interleaved
        emit_va_setup(0)
        g = 0
        for half in range(2):
            for tt in range(TT):
                emit_Cgroup(0, tt, half)
                if tt in (3, 7):
                    emit_Bgroup(0, 0, g // 2, g % 2)
                    g += 1

        if level <= 2:
            for tt in range(TT):
                nc.sync.dma_start(
                    out[0, ts(tt, 128), 0:390], va_tiles[(0, tt)][:, :].bitcast(F32)
                )
            qT, kTt = cur_qk[0]
            nc.sync.dma_start(out[0, 0:128, 390:646], qT[:, 0:512].bitcast(F32))
        else:
            # D0: attention batch 0; fillers = B0(p+1), then xt1/C1/B1(0)
            for p in range(NPAIR):
                if p < NPAIR - 1:
                    for g in reversed(range(4)):
                        pending.appendleft(
                            (800, lambda g=g, pn=p + 1: emit_Bgroup(0, pn, g // 2, g % 2))
                        )
                if p == 0 and level >= 5:
                    # batch-1 inputs early: C1 groups fill D0 pairs 1-4
                    # (fp8 shrank the B fillers below the exp-window need)
                    l_alls[1] = lpool.tile([10, T], F32, tag="l_all", name="l_all")
                    l5s[1] = lpool.tile([2, T], F32, tag="l5", name="l5")
                    pending.append((200, lambda: emit_xt_dma(1)))
                    pending.append((300, lambda: emit_va_setup(1)))
                if p == 2 and level >= 5:
                    for tt in range(TT):
                        for half in range(2):
                            pending.append(
                                (1100, lambda tt=tt, half=half: emit_Cgroup(1, tt, half))
                            )
                emit_D_pair(0, p, 0)
                emit_D_pair(0, p, 1)
                if p == NPAIR - 2:
                    # pairs 0-4 l rows have landed; reciprocal them now so
                    # norm0(p<5) can fill pair 5's exp window, and B1(0,1)
                    # give pair 5 matmul filler
                    emit_recip_a(0)
                    for pn in range(NPAIR - 1):
                        pending.append((350, lambda pn=pn: emit_norm(0, pn, 0)))
                    for pn in range(NPAIR - 1):
                        pending.append((350, lambda pn=pn: emit_norm(0, pn, 1)))
                    if level >= 5:
                        for g in range(4):
                            pending.append(
                                (800, lambda g=g: emit_Bgroup(1, 0, g // 2, g % 2))
                            )
                        for g in range(4):
                            pending.append(
                                (800, lambda g=g: emit_Bgroup(1, 1, g // 2, g % 2))
                            )
            drain_fillers()
            emit_recip_b(0, 0)
            emit_recip_b(0, 1)

            if level <= 4:
                emit_norm(0, NPAIR - 1, 0)
                emit_norm(0, NPAIR - 1, 1)
                for p in range(NPAIR):
                    nc.sync.dma_start(
                        out[0, ts(p, 128), 0:512], yraws[(0, p)][:, :].bitcast(F32)
                    )
                nc.sync.dma_start(
                    out[0, ds(6 * 128, 12), 0:768], l_alls[0][:, 0:768]
                )
            else:
                # D1 fillers: norm0(5)/E0 queued up front, B1(p+1) per pair,
                # norm1 + early E1 in the pair-5 windows
                pending.append((350, lambda: emit_norm(0, NPAIR - 1, 0)))
                pending.append((350, lambda: emit_norm(0, NPAIR - 1, 1)))
                for tt in range(TT):
                    for half in range(2):
                        pending.append(
                            (1100, lambda tt=tt, half=half: emit_Egroup(0, tt, half))
                        )
                for p in range(NPAIR):
                    if 0 < p < NPAIR - 1:
                        # front of the queue: must drain within this pair's
                        # pops so cur_qk[p+1] exists at pair p+1 emission
                        for g in reversed(range(4)):
                            pending.appendleft(
                                (800, lambda g=g, pn=p + 1: emit_Bgroup(1, pn, g // 2, g % 2))
                            )
                    emit_D_pair(1, p, 0)
                    if p == NPAIR - 1:
                        # between pair-5 halves: invert l5 cols 0-511 so
                        # norm1(5, h0) and E1(tt 0-3) can fill qc1's window
                        emit_recip_b(1, 0)
                        pending.append((350, lambda: emit_norm(1, NPAIR - 1, 0)))
                        for tt in range(4):
                            for half in range(2):
                                pending.append(
                                    (1100, lambda tt=tt, half=half: emit_Egroup(1, tt, half))
                                )
                    emit_D_pair(1, p, 1)
                    if p == NPAIR - 2:
                        emit_recip_a(1)
                        for pn in range(NPAIR - 1):
                            pending.append((350, lambda pn=pn: emit_norm(1, pn, 0)))
                        for pn in range(NPAIR - 1):
                            pending.append((350, lambda pn=pn: emit_norm(1, pn, 1)))
                drain_fillers()
                emit_recip_b(1, 1)
                emit_norm(1, NPAIR - 1, 1)
                for tt in range(4, TT):
                    for half in range(2):
                        emit_Egroup(1, tt, half)

    if split_waits:
        split_multi_waits(nc)
    return nc


def make_in_maps(x, W_qkv, b_qkv, W_proj, b_proj):
    x = np.ascontiguousarray(np.asarray(x, dtype=np.float32))
    W_qkv = np.ascontiguousarray(np.asarray(W_qkv, dtype=np.float32))
    b_qkv = np.asarray(b_qkv, dtype=np.float32)
    W_proj = np.ascontiguousarray(np.asarray(W_proj, dtype=np.float32))
    b_proj = np.asarray(b_proj, dtype=np.float32)

    import ml_dtypes

    x16 = x.astype(ml_dtypes.bfloat16)
    wqkv16 = np.ascontiguousarray(W_qkv.astype(ml_dtypes.bfloat16))
    wproj16 = np.ascontiguousarray(W_proj.astype(ml_dtypes.bfloat16))
    bqkt = np.ascontiguousarray(b_qkv[: 2 * C].reshape(2 * NPAIR, 128).T)
    bvbc = np.ascontiguousarray(np.tile(b_qkv[2 * C :].reshape(1, C), (128, 1)))
    bobc = np.ascontiguousarray(np.tile(b_proj.reshape(1, C), (128, 1)))
    m1 = np.triu(np.ones((128, 128), dtype=np.float32))
    maskb = np.ascontiguousarray(
        np.concatenate([m1, m1], axis=1).astype(ml_dtypes.bfloat16)
    )
    # selector: rows j<12, sel[j, 64h+m] = (j == h); fp32 bits used as f32r
    sel = np.zeros((128, C), dtype=np.float32)
    for h in range(H):
        sel[h, 64 * h : 64 * (h + 1)] = 1.0
    sel = sel.astype(ml_dtypes.bfloat16)

    shared = {
        "wqkv": wqkv16,
        "wproj": wproj16,
        "bqkt": bqkt,
        "bvbc": bvbc,
        "bobc": bobc,
        "maskb": maskb,
        "sel12": sel,
    }
    in_maps = []
    for c in range(NCORES):
        m = dict(shared)
        xc = x16[B_LOC * c : B_LOC * (c + 1)]  # [B_LOC, T, C]
        # xt[b, p, k, t] = x[b, t, 128k+p]
        xtc = xc.transpose(0, 2, 1).reshape(B_LOC, KT, 128, T).transpose(0, 2, 1, 3)
        m["xt"] = np.ascontiguousarray(xtc)
        in_maps.append(m)
    return in_maps


_PROGRAM = None


def kernel(x, W_qkv, b_qkv, W_proj, b_proj):
    global _PROGRAM
    if _PROGRAM is None:
        _PROGRAM = build_program()
    in_maps = make_in_maps(x, W_qkv, b_qkv, W_proj, b_proj)
    res = run_bass_kernel_spmd(_PROGRAM, in_maps, list(range(NCORES)))
    out = np.concatenate([res.results[c]["out"] for c in range(NCORES)], axis=0)
    return out.astype(np.float32)


if __name__ == "__main__":
    nc = build_program()
    print("built ok; instructions:", sum(len(bb.instructions) for f in nc.m.functions for bb in f.blocks))
